# revision 1
# baseline (speedup 1.0000x reference)
"""Trainium2 Bass kernel for the attention-LSTM decoder (nn_Decoder).

Sharding: data-parallel over batch, 8 batch elements per core, weights
replicated, no collectives. Per core:
  phase A: embedding gather (indirect DMA) + pregemm of the x-dependent
           gate term xW = emb[tok] @ Wx.T + bias  (stored in DRAM, f32)
  phase B: 128 sequential steps: gates matmul (weights stationary,
           gates.T layout), LSTM pointwise, attention scores via
           col-tiled M=1 matmuls (4 concurrent streams), softmax on
           ACT/DVE, weighted sum likewise col-tiled, MLP1 -> z_t
  phase C: deferred vocab projection Z @ W2.T + b2 (batched over all
           128*8 rows)
All matmul operands bf16, accumulation f32.
"""

import numpy as np
import ml_dtypes

V, E, H, B, TO, TI = 10000, 512, 512, 64, 128, 1000
NCORES = 8
BL = B // NCORES          # local batch = 8
G = 4 * H                 # 2048 gate rows
KC = 2 * H                # 1024 contraction for [att|h]
TT = 8                    # t-tiles of 128 (1000 padded to 1024)
BFNP = ml_dtypes.bfloat16

_cache = {}


def _build(n_steps, dyn_loop, dbg=False, loop_reps=1, pc_reps=1, abl=(), unroll=2, pc_dma=500):
    from concourse import bacc
    import concourse.bass as bass
    import concourse.mybir as mybir
    import concourse.tile as tile
    from concourse.masks import make_identity

    f32, bf16, i32 = mybir.dt.float32, mybir.dt.bfloat16, mybir.dt.int32
    ET = mybir.EngineType

    nc = bacc.Bacc("TRN2", target_bir_lowering=False, debug=False)

    tok = nc.dram_tensor("tok", [TO // 16, 128, 1], i32, kind="ExternalInput")
    emb_bf = nc.dram_tensor("emb_bf", [V, E], bf16, kind="ExternalInput")
    encT = nc.dram_tensor("encT", [128, BL, 4, TI], bf16, kind="ExternalInput")
    encB = nc.dram_tensor("encB", [128, BL, TT, H], bf16, kind="ExternalInput")
    Wah = nc.dram_tensor("Wah", [128, 8, G], bf16, kind="ExternalInput")
    WxT = nc.dram_tensor("WxT", [128, 4, G], bf16, kind="ExternalInput")
    W1T = nc.dram_tensor("W1T", [128, 8, H], bf16, kind="ExternalInput")
    W2T = nc.dram_tensor("W2T", [128, 4, V], bf16, kind="ExternalInput")
    biasg = nc.dram_tensor("biasg", [128, 16], f32, kind="ExternalInput")
    b1d = nc.dram_tensor("b1d", [128, 4], f32, kind="ExternalInput")
    b2d = nc.dram_tensor("b2d", [128, V], f32, kind="ExternalInput")
    out = nc.dram_tensor("out", [n_steps * BL, V], f32, kind="ExternalOutput")
    xWT_d = nc.dram_tensor("xWT_d", [128, 16, TO, BL], f32)
    if dbg:
        d_gates = nc.dram_tensor("d_gates", [128, 128], f32, kind="ExternalOutput")
        d_hT = nc.dram_tensor("d_hT", [128, 4, BL], f32, kind="ExternalOutput")
        d_cT = nc.dram_tensor("d_cT", [128, 4, BL], f32, kind="ExternalOutput")
        d_wexp = nc.dram_tensor("d_wexp", [128, 2000], f32, kind="ExternalOutput")
        d_wc8 = nc.dram_tensor("d_wc8", [8, 1024], f32, kind="ExternalOutput")
        d_wTs = nc.dram_tensor("d_wTs", [128, 64], f32, kind="ExternalOutput")
        d_attn = nc.dram_tensor("d_attn", [128, 1024], f32, kind="ExternalOutput")
        d_attc = nc.dram_tensor("d_attc", [8, 512], f32, kind="ExternalOutput")
        d_aT = nc.dram_tensor("d_aT", [128, 4, BL], f32, kind="ExternalOutput")
        d_xw = nc.dram_tensor("d_xw", [128, 16, 2, BL], f32, kind="ExternalOutput")
        d_zT = nc.dram_tensor("d_zT", [128, 4, 2, BL], f32, kind="ExternalOutput")

    with tile.TileContext(nc) as tc:
      with tc.tile_pool(name="wp", bufs=1) as wp:
        ident = wp.tile([128, 128], bf16, tag="ident")
        make_identity(nc, ident[:])
        zT_all = wp.tile([128, 4, TO, BL], bf16, tag="zT")

        # enc/weights/state pool — released before phase C to free SBUF
        with tc.tile_pool(name="we", bufs=1) as we:
            wp = we
            Wah_sb = wp.tile([128, 8, G], bf16, tag="Wah")
            nc.sync.dma_start(Wah_sb[:], Wah[:])
            W1_sb = wp.tile([128, 8, H], bf16, tag="W1")
            nc.sync.dma_start(W1_sb[:], W1T[:])
            encT_sb = wp.tile([128, BL, 4, TI], bf16, tag="encT")
            nc.sync.dma_start(encT_sb[:], encT[:])
            encB_sb = wp.tile([128, BL, TT, H], bf16, tag="encB")
            nc.sync.dma_start(encB_sb[:], encB[:])
            biasg_sb = wp.tile([128, 16], f32, tag="biasg")
            nc.sync.dma_start(biasg_sb[:], biasg[:])
            b1_sb = wp.tile([128, 4], f32, tag="b1")
            nc.sync.dma_start(b1_sb[:], b1d[:])

            # persistent state (ping-pong pairs)
            hT = [wp.tile([128, 4, BL], bf16, tag=f"hT{k}", name=f"hT{k}") for k in range(2)]
            cT = [wp.tile([128, 4, BL], f32, tag=f"cT{k}", name=f"cT{k}") for k in range(2)]
            aT = [wp.tile([128, 4, BL], bf16, tag=f"aT{k}", name=f"aT{k}") for k in range(2)]
            wc8 = wp.tile([8, 1024], bf16, tag="wc8")
            attc = wp.tile([8, 512], bf16, tag="attc")
            nc.gpsimd.memset(wc8[:], 0.0)
            nc.gpsimd.memset(hT[0][:], 0.0)
            nc.gpsimd.memset(cT[0][:], 0.0)
            nc.gpsimd.memset(aT[0][:], 0.0)

            # ---------------- phase A: embed + pregemm ----------------
            with (
                tc.tile_pool(name="pa", bufs=2) as pa,
                tc.tile_pool(name="paps", bufs=2, space="PSUM") as paps,
            ):
                XT_sb = pa.tile([128, 4, 1024], bf16, tag="XT", bufs=1)
                for j in range(8):
                    idx_t = pa.tile([128, 1], i32, tag="idx")
                    nc.sync.dma_start(idx_t[:], tok[j, :, :])
                    Xb = pa.tile([128, E], bf16, tag="Xb")
                    nc.gpsimd.indirect_dma_start(
                        out=Xb[:], out_offset=None, in_=emb_bf[:],
                        in_offset=bass.IndirectOffsetOnAxis(ap=idx_t[:, :1], axis=0),
                    )
                    for e in range(4):
                        xt_ps = paps.tile([128, 128], bf16, tag="xtp")
                        nc.tensor.transpose(xt_ps[:], Xb[:, 128 * e:128 * e + 128], ident[:])
                        nc.vector.tensor_copy(XT_sb[:, e, 128 * j:128 * j + 128], xt_ps[:])
                for m in range(16):
                    wx_t = pa.tile([128, 4, 128], bf16, tag="wx", bufs=3)
                    nc.sync.dma_start(wx_t[:], WxT[:, :, 128 * m:128 * m + 128])
                    for ch in range(2):
                        pg = paps.tile([128, 512], f32, tag="pg")
                        for e in range(4):
                            nc.tensor.matmul(
                                pg[:], lhsT=wx_t[:, e, :],
                                rhs=XT_sb[:, e, 512 * ch:512 * ch + 512],
                                start=(e == 0), stop=(e == 3),
                            )
                        xw_sb = pa.tile([128, 512], f32, tag="xwo", bufs=2)
                        nc.vector.tensor_scalar_add(xw_sb[:], pg[:], biasg_sb[:, m:m + 1])
                        nc.sync.dma_start(
                            xWT_d[:, m, 64 * ch:64 * ch + 64, :],
                            xw_sb[:].rearrange("p (t b) -> p t b", b=BL),
                        )

            # ---------------- phase B: recurrent loop ----------------
            with (
                tc.tile_pool(name="lb", bufs=1) as lb,
                tc.tile_pool(name="lps", bufs=1, space="PSUM") as lps,
            ):
                def step(texpr, src, dst):
                    dbg_wexp = []
                    xw_sb = lb.tile([128, 16, BL], f32, tag="xw", bufs=4)
                    if "no_xw" in abl:
                        nc.vector.memset(xw_sb[:], 0.0)
                    else:
                        nc.sync.dma_start(
                            xw_sb[:],
                            xWT_d[:, :, bass.ds(texpr, 1), :].rearrange("p m one b -> p (one m) b"),
                        )
                    # gates.T accumulate: out cols m*8+b
                    gsb = lb.tile([128, 128], f32, tag="gsb", bufs=2)
                    if "no_gates" in abl:
                        nc.vector.tensor_copy(gsb[:], xw_sb[:].rearrange("p m b -> p (m b)"))
                    else:
                        g_ps = lps.tile([128, 128], f32, tag="g")
                        for m in range(16):
                            for kt in range(8):
                                rhs = aT[src][:, kt, :] if kt < 4 else hT[src][:, kt - 4, :]
                                nc.tensor.matmul(
                                    g_ps[:, 8 * m:8 * m + 8],
                                    lhsT=Wah_sb[:, kt, 128 * m:128 * m + 128],
                                    rhs=rhs, start=(kt == 0), stop=(kt == 7),
                                )
                        nc.vector.tensor_add(
                            gsb[:], g_ps[:], xw_sb[:].rearrange("p m b -> p (m b)")
                        )
                    # pointwise LSTM per h-slice
                    AF = mybir.ActivationFunctionType
                    for hs in range(4):
                        ci = gsb[:, 8 * hs:8 * hs + 8]
                        cf = gsb[:, 8 * (4 + hs):8 * (4 + hs) + 8]
                        cg = gsb[:, 8 * (8 + hs):8 * (8 + hs) + 8]
                        co = gsb[:, 8 * (12 + hs):8 * (12 + hs) + 8]
                        si = lb.tile([128, 8], f32, tag="si", bufs=4)
                        sf = lb.tile([128, 8], f32, tag="sf", bufs=4)
                        tg = lb.tile([128, 8], f32, tag="tg", bufs=4)
                        so = lb.tile([128, 8], f32, tag="so", bufs=4)
                        nc.scalar.activation(si[:], ci, AF.Sigmoid)
                        nc.scalar.activation(sf[:], cf, AF.Sigmoid)
                        nc.scalar.activation(tg[:], cg, AF.Tanh)
                        nc.scalar.activation(so[:], co, AF.Sigmoid)
                        t1 = lb.tile([128, 8], f32, tag="t1", bufs=4)
                        nc.vector.tensor_mul(t1[:], sf[:], cT[src][:, hs, :])
                        t2 = lb.tile([128, 8], f32, tag="t2", bufs=4)
                        nc.vector.tensor_mul(t2[:], si[:], tg[:])
                        nc.vector.tensor_add(cT[dst][:, hs, :], t1[:], t2[:])
                        tc_ = lb.tile([128, 8], f32, tag="tc", bufs=4)
                        nc.scalar.activation(tc_[:], cT[dst][:, hs, :], AF.Tanh)
                        nc.vector.tensor_mul(hT[dst][:, hs, :], so[:], tc_[:])

                    # attention scores, col-tiled M=1 (4 concurrent streams)
                    # bank-aligned: round r at cols 1024r, chunks at +0 and +512
                    if "no_scores" in abl:
                        recips = []
                        for r in range(2):
                            rec = lb.tile([128, 1], f32, tag="rec", bufs=2)
                            nc.vector.memset(rec[:], 1.0)
                            recips.append(rec)
                    else:
                      s_ps = lps.tile([128, 2048], f32, tag="sbig")
                      for r in range(2):
                        for hs in range(4):
                            for ch in range(2):
                                for j in range(4):
                                    b = 4 * r + j
                                    nc.tensor.matmul(
                                        s_ps[32 * j:32 * j + 1,
                                             1024 * r + 512 * ch:1024 * r + 512 * ch + 500],
                                        lhsT=hT[dst][:, hs, b:b + 1],
                                        rhs=encT_sb[:, b, hs, 500 * ch:500 * ch + 500],
                                        start=(hs == 0), stop=(hs == 3),
                                        tile_position=(0, 32 * j),
                                    )
                      # softmax (rows 32j), exp with fused sum
                      recips = []
                      for r in range(2):
                        sview = s_ps[:, 1024 * r:1024 * r + 1024].rearrange(
                            "p (c t) -> p c t", t=512)[:, :, 0:500]
                        nm = lb.tile([128, 1], f32, tag="nm", bufs=2)
                        nc.vector.tensor_reduce(
                            nm[:], sview,
                            axis=mybir.AxisListType.XY, op=mybir.AluOpType.max, negate=True,
                        )
                        wexp = lb.tile([128, 1000], bf16, tag="wexp", bufs=2)
                        sums = lb.tile([128, 1], f32, tag="sums", bufs=2)
                        nc.scalar.activation(
                            wexp[:].rearrange("p (c t) -> p c t", t=500), sview,
                            mybir.ActivationFunctionType.Exp,
                            bias=nm[:], scale=1.0, accum_out=sums[:],
                        )
                        rec = lb.tile([128, 1], f32, tag="rec", bufs=2)
                        nc.vector.reciprocal(rec[:], sums[:])
                        recips.append(rec)
                        dbg_wexp.append(wexp)
                        nc.sync.dma_start(wc8[4 * r:4 * r + 4, 0:1000], wexp[0:128:32, :])
                    # w~ transpose -> [t, b] layout
                    wTs = lb.tile([128, 64], bf16, tag="wTs", bufs=2)
                    if "no_wT" in abl:
                        nc.vector.memset(wTs[:], 0.0)
                    else:
                        wT_ps = lps.tile([128, 64], bf16, tag="tiny")
                        for tt in range(8):
                            nc.tensor.transpose(
                                wT_ps[:, 8 * tt:8 * tt + 8],
                                wc8[:, 128 * tt:128 * tt + 128], ident[0:8, 0:8],
                            )
                        nc.vector.tensor_copy(wTs[:], wT_ps[:])
                    # weighted sum, col-tiled M=1
                    if "no_wsum" in abl:
                        nc.vector.memset(aT[dst][:].rearrange("p a b -> p (a b)"), 0.0)
                        attn = None
                    else:
                        a_ps = lps.tile([128, 1024], f32, tag="med")
                        for r in range(2):
                            for tt in range(8):
                                for j in range(4):
                                    b = 4 * r + j
                                    nc.tensor.matmul(
                                        a_ps[32 * j:32 * j + 1, 512 * r:512 * r + 512],
                                        lhsT=wTs[:, 8 * tt + b:8 * tt + b + 1],
                                        rhs=encB_sb[:, b, tt, :],
                                        start=(tt == 0), stop=(tt == 7),
                                        tile_position=(0, 32 * j),
                                    )
                        # normalize + compact + transpose att
                        attn = lb.tile([128, 1024], bf16, tag="attn", bufs=2)
                        for r in range(2):
                            nc.scalar.activation(
                                attn[:, 512 * r:512 * r + 512], a_ps[:, 512 * r:512 * r + 512],
                                mybir.ActivationFunctionType.Copy, bias=0.0, scale=recips[r][:],
                            )
                            nc.sync.dma_start(
                                attc[4 * r:4 * r + 4, :], attn[0:128:32, 512 * r:512 * r + 512]
                            )
                        at_ps = lps.tile([128, 32], bf16, tag="tiny")
                        for hs in range(4):
                            nc.tensor.transpose(
                                at_ps[:, 8 * hs:8 * hs + 8],
                                attc[:, 128 * hs:128 * hs + 128], ident[0:8, 0:8],
                            )
                        nc.vector.tensor_copy(
                            aT[dst][:].rearrange("p a b -> p (a b)"), at_ps[:]
                        )
                    # MLP1 -> zT
                    if "no_mlp" in abl:
                        return
                    z_ps = lps.tile([128, 32], f32, tag="tiny")
                    for ms in range(4):
                        for kt in range(8):
                            rhs = hT[dst][:, kt, :] if kt < 4 else aT[dst][:, kt - 4, :]
                            nc.tensor.matmul(
                                z_ps[:, 8 * ms:8 * ms + 8],
                                lhsT=W1_sb[:, kt, 128 * ms:128 * ms + 128],
                                rhs=rhs, start=(kt == 0), stop=(kt == 7),
                            )
                    for ms in range(4):
                        nc.scalar.activation(
                            zT_all[:, ms, bass.ds(texpr, 1), :],
                            z_ps[:, 8 * ms:8 * ms + 8].rearrange("p (one b) -> p one b", one=1),
                            mybir.ActivationFunctionType.Tanh, bias=b1_sb[:, ms:ms + 1],
                        )
                    if dbg and isinstance(texpr, int) and texpr == 0:
                        def cvt(dst, src):
                            tmp = lb.tile(list(src.shape), f32, tag="dbgt", bufs=1, name="dbgt")
                            nc.vector.tensor_copy(tmp[:], src)
                            nc.sync.dma_start(dst, tmp[:])
                        nc.sync.dma_start(d_gates[:], gsb[:])
                        cvt(d_hT[:], hT[dst][:])
                        nc.sync.dma_start(d_cT[:], cT[dst][:])
                        cvt(d_wTs[:], wTs[:])
                        cvt(d_attn[:], attn[:])
                        cvt(d_wexp[:, 0:1000], dbg_wexp[0][:])
                        cvt(d_wexp[:, 1000:2000], dbg_wexp[1][:])
                        cvt(d_aT[:], aT[dst][:])

                if dyn_loop:
                    for _rep in range(loop_reps):
                        with tc.For_i(0, n_steps // unroll,
                                      hint_engines=(ET.PE, ET.Activation, ET.DVE, ET.SP, ET.Pool)) as iv:
                            for u in range(unroll):
                                step(iv * unroll + u, u % 2, 1 - u % 2)
                else:
                    for t in range(n_steps):
                        step(t, t % 2, 1 - t % 2)

        if dbg:
            nc.sync.dma_start(d_xw[:], xWT_d[:, :, 0:2, :])
            with tc.tile_pool(name="dz", bufs=1) as dz:
                zt = dz.tile([128, 4, 2, BL], f32, tag="zt")
                nc.vector.tensor_copy(zt[:], zT_all[:, :, 0:2, :])
                nc.sync.dma_start(d_zT[:], zt[:])

        # ---------------- phase C: vocab projection ----------------
        with (
            tc.tile_pool(name="pc", bufs=1) as pc,
            tc.tile_pool(name="pcps", bufs=3, space="PSUM") as pcps,
        ):
            W2_sb = pc.tile([128, 4, V], bf16, tag="W2")
            nc.sync.dma_start(W2_sb[:], W2T[:])
            b2_sb = pc.tile([128, V], f32, tag="b2")
            nc.sync.dma_start(b2_sb[:], b2d[:])
            n_zm = (n_steps * BL) // 128
            grp = pc_dma // 500          # vc chunks per DMA batch
            for zm in list(range(n_zm)) * pc_reps:
                for vg in range(20 // grp):
                    o_sb = pc.tile([128, grp * 500], f32, tag="osb", bufs=4, name="osb")
                    for vsub in range(grp):
                        vc = vg * grp + vsub
                        o_ps = pcps.tile([128, 500], f32, tag="o", bufs=3, name="o_ps")
                        for kt in range(4):
                            nc.tensor.matmul(
                                o_ps[:],
                                lhsT=zT_all[:, kt, 16 * zm:16 * zm + 16, :].rearrange("p t b -> p (t b)"),
                                rhs=W2_sb[:, kt, 500 * vc:500 * vc + 500],
                                start=(kt == 0), stop=(kt == 3),
                            )
                        nc.vector.tensor_add(
                            o_sb[:, 500 * vsub:500 * vsub + 500], o_ps[:],
                            b2_sb[:, 500 * vc:500 * vc + 500])
                    nc.sync.dma_start(
                        out[128 * zm:128 * zm + 128, grp * 500 * vg:grp * 500 * (vg + 1)],
                        o_sb[:])

    nc.compile()
    return nc


def _host_prep(inputs, n_steps):
    """Build per-core input maps with all layout/casting prep on host."""
    tok = np.asarray(inputs["padded_input"])            # (64, 128) i32
    enc = np.asarray(inputs["encoder_padded_outputs"], np.float32)  # (64, 1000, 512)
    emb = np.asarray(inputs["emb"], np.float32)
    W_ih = np.asarray(inputs["W_ih"], np.float32)
    b_ih = np.asarray(inputs["b_ih"], np.float32)
    W_hh = np.asarray(inputs["W_hh"], np.float32)
    b_hh = np.asarray(inputs["b_hh"], np.float32)
    W1 = np.asarray(inputs["W1"], np.float32)
    b1 = np.asarray(inputs["b1"], np.float32)
    W2 = np.asarray(inputs["W2"], np.float32)
    b2 = np.asarray(inputs["b2"], np.float32)

    emb_bf = np.ascontiguousarray(emb.astype(BFNP))
    Wahm = np.concatenate([W_ih[:, E:], W_hh], axis=1)   # (G, 1024)
    Wah_l = np.ascontiguousarray(Wahm.T.reshape(8, 128, G).transpose(1, 0, 2).astype(BFNP))
    WxT_l = np.ascontiguousarray(W_ih[:, :E].T.reshape(4, 128, G).transpose(1, 0, 2).astype(BFNP))
    W1T_l = np.ascontiguousarray(W1.T.reshape(8, 128, H).transpose(1, 0, 2).astype(BFNP))
    W2T_l = np.ascontiguousarray(W2.T.reshape(4, 128, V).transpose(1, 0, 2).astype(BFNP))
    biasg_l = np.ascontiguousarray((b_ih + b_hh).reshape(16, 128).T.astype(np.float32))
    b1_l = np.ascontiguousarray(b1.reshape(4, 128).T.astype(np.float32))
    b2_l = np.ascontiguousarray(np.broadcast_to(b2[None, :], (128, V)).astype(np.float32))

    shared = {
        "emb_bf": emb_bf, "Wah": Wah_l, "WxT": WxT_l, "W1T": W1T_l,
        "W2T": W2T_l, "biasg": biasg_l, "b1d": b1_l, "b2d": b2_l,
    }
    in_maps = []
    for k in range(NCORES):
        bs = slice(BL * k, BL * (k + 1))
        tk = tok[bs]                                     # (8, 128)
        # tok layout: [TO//16, 128, 1] with partition p = (t-16j)*8 + b
        tok_l = np.ascontiguousarray(
            tk.T.reshape(TO // 16, 16 * BL, 1).astype(np.int32)
        )
        encl = enc[bs]                                   # (8, 1000, 512)
        # encT: [p, b, hs, t] = enc[b, t, 128hs+p]
        encT_l = np.ascontiguousarray(
            encl.astype(BFNP).transpose(2, 0, 1).reshape(4, 128, BL, TI).transpose(1, 2, 0, 3)
        )
        # encB: [p, b, tt, h] = enc[b, 128tt+p, h], zero-padded to 1024 t
        encp = np.zeros((BL, TT * 128, H), BFNP)
        encp[:, :TI] = encl.astype(BFNP)
        encB_l = np.ascontiguousarray(
            encp.reshape(BL, TT, 128, H).transpose(2, 0, 1, 3)
        )
        in_maps.append({"tok": tok_l, "encT": encT_l, "encB": encB_l, **shared})
    return in_maps


def kernel(**inputs) -> np.ndarray:
    from concourse.bass_utils import run_bass_kernel_spmd

    n_steps = TO
    key = (n_steps, True)
    if key not in _cache:
        _cache[key] = _build(n_steps, dyn_loop=True)
    nc = _cache[key]
    in_maps = _host_prep(inputs, n_steps)
    res = run_bass_kernel_spmd(nc, in_maps, core_ids=list(range(NCORES)))
    # unshard: per-core out rows are (t*8 + b_local) -> full (b, t)
    full = np.empty((B, TO, V), np.float32)
    for k in range(NCORES):
        o = res.results[k]["out"].reshape(TO, BL, V)
        full[BL * k:BL * (k + 1)] = o.transpose(1, 0, 2)
    return full.reshape(B * TO, V)


def _make_runner(nc, in_maps, reps=1):
    """Build a reusable sharded PJRT callable with device-resident inputs.

    Mirrors bass2jax.run_bass_via_pjrt multi-core path, but returns a
    closure that re-executes without re-transferring inputs.
    """
    import jax
    import numpy as np
    from jax.sharding import Mesh, PartitionSpec, NamedSharding
    from jax.experimental.shard_map import shard_map
    from concourse import bass2jax, mybir
    bass2jax.install_neuronx_cc_hook()
    n_cores = len(in_maps)

    partition_name = nc.partition_id_tensor.name if nc.partition_id_tensor else None
    in_names, out_names, out_avals, zero_outs = [], [], [], []
    for alloc in nc.m.functions[0].allocations:
        if not isinstance(alloc, mybir.MemoryLocationSet):
            continue
        name = alloc.memorylocations[0].name
        if alloc.kind == "ExternalInput":
            if name != partition_name:
                in_names.append(name)
        elif alloc.kind == "ExternalOutput":
            out_names.append(name)
            shape = tuple(alloc.tensor_shape)
            dtype = mybir.dt.np(alloc.dtype)
            out_avals.append(jax.core.ShapedArray(shape, dtype))
            zero_outs.append(np.zeros(shape, dtype))
    n_params = len(in_names)
    all_in = in_names + out_names
    if partition_name is not None:
        all_in.append(partition_name)

    def _body(*args):
        ins = list(args[:n_params])
        zo = list(args[n_params:])
        acc = None
        for _ in range(reps):
            operands = ins + zo
            if partition_name is not None:
                operands.append(bass2jax.partition_id_tensor())
            outs = list(bass2jax._bass_exec_p.bind(
                *operands, out_avals=tuple(out_avals), in_names=tuple(all_in),
                out_names=tuple(out_names), lowering_input_output_aliases=(),
                sim_require_finite=True, sim_require_nnan=True, nc=nc))
            acc = outs if acc is None else [a + o for a, o in zip(acc, outs)]
        return tuple(acc)

    devices = jax.devices()[:n_cores]
    mesh = Mesh(np.asarray(devices), ("core",))
    spec = NamedSharding(mesh, PartitionSpec("core"))
    fn = jax.jit(
        shard_map(_body, mesh=mesh,
                  in_specs=(PartitionSpec("core"),) * (n_params + len(out_names)),
                  out_specs=(PartitionSpec("core"),) * len(out_names),
                  check_rep=False),
        keep_unused=True)
    concat_in = [
        jax.device_put(np.concatenate([np.asarray(m[nm]) for m in in_maps], axis=0), spec)
        for nm in in_names
    ]
    concat_zeros = [
        jax.device_put(np.zeros((n_cores * z.shape[0], *z.shape[1:]), z.dtype), spec)
        for z in zero_outs
    ]
    jax.block_until_ready(concat_in)
    jax.block_until_ready(concat_zeros)

    def run():
        out = fn(*concat_in, *concat_zeros)
        jax.block_until_ready(out)
        return out
    return run, out_names, out_avals


def bench(inputs, iters=6):
    """Return (best_exec_seconds, baseline_seconds) via repeated device runs."""
    import time
    nc = _cache.get((TO, True)) or _build(TO, dyn_loop=True)
    _cache[(TO, True)] = nc
    in_maps = _host_prep(inputs, TO)
    run, _, _ = _make_runner(nc, in_maps)
    run()  # warm
    times = []
    for _ in range(iters):
        t0 = time.perf_counter()
        run()
        times.append(time.perf_counter() - t0)
    # baseline: trivial kernel, same dispatch path
    base = _trivial_baseline()
    return min(times), base, times


_base_cache = {}


def _trivial_baseline():
    import time
    import numpy as np
    if "t" not in _base_cache:
        from concourse import bacc
        import concourse.mybir as mybir
        import concourse.tile as tile
        nc = bacc.Bacc("TRN2", target_bir_lowering=False, debug=False)
        x = nc.dram_tensor("x", [128, 128], mybir.dt.float32, kind="ExternalInput")
        y = nc.dram_tensor("y", [128, 128], mybir.dt.float32, kind="ExternalOutput")
        with tile.TileContext(nc) as tc:
            with tc.tile_pool(name="sb", bufs=2) as sb:
                t = sb.tile([128, 128], mybir.dt.float32)
                nc.sync.dma_start(t[:], x[:])
                nc.scalar.mul(t[:], t[:], 2.0)
                nc.sync.dma_start(y[:], t[:])
        nc.compile()
        maps = [{"x": np.zeros((128, 128), np.float32)} for _ in range(NCORES)]
        run, _, _ = _make_runner(nc, maps)
        _base_cache["t"] = run
    run = _base_cache["t"]
    run()
    times = []
    for _ in range(4):
        t0 = time.perf_counter()
        run()
        times.append(time.perf_counter() - t0)
    return min(times)

